# revision 4
# baseline (speedup 1.0000x reference)
"""Binary-conv BasicBlock (sign-act 3x3 binary conv + BN(eval) + residual).

Full shapes: x (32,128,56,56) f32, weight (128,128,3,3), BN params (128,).
Strategy: data-parallel over batch N across 8 NeuronCores (4 images/core).
Per image on-device:
  - sign(x) on ScalarE into a zero-padded fp8e4 tile (58x58 rows, flat)
  - conv = 9 taps as 4 DoubleRow fp8 matmuls (2 taps each, K_eff=256) + 1
    normal fp8 matmul, accumulating in PSUM.  +/-1 is exact in fp8e4 and the
    integer partial sums (<=1152) are exact in fp32 PSUM -> conv bit-exact.
    Matmuls compute 58-wide "extended rows" (N=406 = 7 rows x 58, contiguous)
    so the DoubleRow moving operand stays a 3D [K][2][flat] AP; the 2 pad
    columns per row are garbage and simply never read by the epilogue.
  - epilogue on VectorE: out = (psum * s) + (x + t) via scalar_tensor_tensor;
    (x + t) precomputed per image on otherwise-idle GpSimd.
  - chunk-major over 4 PSUM pair-tiles (2 banks each) -> epilogue and stores
    overlap the next chunk's matmuls; per-pair DMA stores.
  - x loads split in halves w/ split sign for fast pipeline start; fp8 warmup
    matmuls keep PE HAM un-throttled through the initial DMA wait.
"""

import numpy as np
import ml_dtypes

_N, _C, _H, _W = 32, 128, 56, 56
_P = 128
_NCORES = 8
_NPI = _N // _NCORES  # images per core
_HP, _WP = _H + 2, _W + 2
_NPIX = _H * _W
_APAD = _HP * _WP + 4  # flat padded sign tile + slack for extended-row reads
_BN_EPS = 1e-5
_CH = 7               # output rows per PSUM bank chunk
_NCH = _H // _CH      # 8 chunks per image
_NPAIR = _NCH // 2    # 4 psum pair-tiles (2 banks each) per image
_CN = _CH * _W        # 392 valid elems per chunk
_XN = _CH * _WP       # 406 extended-row elems per chunk matmul

# tap order t = kh*3 + kw; DoubleRow pairs + final single tap
_PAIRS = [(0, 1), (2, 3), (4, 5), (6, 7)]
_SINGLE = 8
_TAPOFF = [58 * (t // 3) + (t % 3) for t in range(9)]

_cache = {}


def _build_program():
    import concourse.bass as bass
    import concourse.bacc as bacc
    import concourse.mybir as mybir
    import concourse.tile as tile

    f32 = mybir.dt.float32
    fp8 = mybir.dt.float8e4

    nc = bacc.Bacc("TRN2", target_bir_lowering=False, debug=False)

    x_d = nc.dram_tensor("x", [_NPI, _C, _NPIX], f32, kind="ExternalInput")
    wdr_d = nc.dram_tensor("wdr", [_C, 4, 2, _P], fp8, kind="ExternalInput")
    w8_d = nc.dram_tensor("w8", [_C, _P], fp8, kind="ExternalInput")
    s_d = nc.dram_tensor("s", [_P, 1], f32, kind="ExternalInput")
    t_d = nc.dram_tensor("t", [_P, 1], f32, kind="ExternalInput")
    o_d = nc.dram_tensor("o", [_NPI, _P, _NPIX], f32, kind="ExternalOutput")

    SIGN = mybir.ActivationFunctionType.Sign
    DR = mybir.MatmulPerfMode.DoubleRow
    MULT, ADD = mybir.AluOpType.mult, mybir.AluOpType.add
    HROWS = _H // 2  # 28 rows per x half

    with tile.TileContext(nc) as tc:
        with (
            tc.tile_pool(name="const", bufs=1) as cpool,
            tc.tile_pool(name="xin", bufs=3) as xpool,
            tc.tile_pool(name="apad", bufs=1) as apool,
            tc.tile_pool(name="outp", bufs=4) as opool,
            tc.tile_pool(name="ps", bufs=4, space="PSUM") as pspool,
        ):
            # Warmup source: tiny zero tile; matmuls on it keep the PE busy
            # (HAM stays at 8/8) while the first image loads.
            dummy = cpool.tile([_C, _P], fp8)
            nc.any.memset(dummy[:], 0.0)

            x_tiles = [None] * _NPI
            xp_tiles = [None] * _NPI

            def load_x(n):
                x_t = xpool.tile([_C, _NPIX], f32, name="x_t", tag="x")
                for h in range(2):
                    nc.sync.dma_start(
                        x_t[:, h * HROWS * _W : (h + 1) * HROWS * _W],
                        x_d[n, :, h * HROWS * _W : (h + 1) * HROWS * _W],
                    )
                x_tiles[n] = x_t

            load_x(0)

            wdr_t = cpool.tile([_C, 4, 2, _P], fp8)
            nc.sync.dma_start(wdr_t[:], wdr_d[:])
            w8_t = cpool.tile([_C, _P], fp8)
            nc.sync.dma_start(w8_t[:], w8_d[:])
            s_t = cpool.tile([_P, 1], f32)
            nc.sync.dma_start(s_t[:], s_d[:])
            t_t = cpool.tile([_P, 1], f32)
            nc.sync.dma_start(t_t[:], t_d[:])

            # Two persistent padded sign tiles; zeroed once (borders + slack
            # stay 0), only the 56x56 interior is rewritten per image.
            a_tiles = []
            for i in range(2):
                a_t = apool.tile([_C, _APAD], fp8, name=f"apad{i}", tag=f"apad{i}")
                nc.any.memset(a_t[:], 0.0)
                a_tiles.append(a_t)

            def sign_img(n):
                x_v = x_tiles[n][:].rearrange("c (h w) -> c h w", h=_H)
                a_v = a_tiles[n % 2][:, : _HP * _WP].rearrange(
                    "c (h w) -> c h w", w=_WP
                )
                for h in range(2):
                    r = h * HROWS
                    nc.scalar.activation(
                        a_v[:, 1 + r : 1 + r + HROWS, 1 : _W + 1],
                        x_v[:, r : r + HROWS, :],
                        SIGN,
                    )

            def xplust(n):
                xp = xpool.tile([_C, _NPIX], f32, name="xp_t", tag="xp")
                nc.gpsimd.tensor_scalar_add(xp[:], x_tiles[n][:], t_t[:, 0:1])
                xp_tiles[n] = xp

            sign_img(0)
            xplust(0)

            # PE warmup while image-0 DMA+sign are in flight (start/stop=True;
            # results discarded when the real group restarts the bank).
            warm_ps = pspool.tile([_P, 2, 512], f32, name="warm_ps", tag="ps")
            for i in range(72):
                nc.tensor.matmul(
                    warm_ps[:, i % 2, :128],
                    dummy[:],
                    dummy[:],
                    start=True,
                    stop=True,
                )

            def dr_rhs(a_t, r0, q):
                t0, t1 = _PAIRS[q]
                o0, o1 = _TAPOFF[t0], _TAPOFF[t1]
                base = a_t[:, 0:_XN]  # borrow partition stride from the tile
                return bass.AP(
                    tensor=base.tensor,
                    offset=int(base.offset) + r0 * _WP + o0,
                    ap=[tuple(base.ap[0]), (o1 - o0, 2), (1, _XN)],
                )

            for n in range(_NPI):
                # Emit next image's load+sign ahead of this image's epilogue
                # so the in-order ScalarE stream never stalls next matmuls.
                if n + 1 < _NPI:
                    load_x(n + 1)
                    sign_img(n + 1)
                    xplust(n + 1)
                a_t = a_tiles[n % 2]

                for p in range(_NPAIR):
                    pst = pspool.tile([_P, 2, 512], f32, name="pst", tag="ps")
                    for b in range(2):
                        c = 2 * p + b
                        r0 = c * _CH
                        for q in range(4):
                            nc.tensor.matmul(
                                pst[:, b, :_XN],
                                wdr_t[:, q],
                                dr_rhs(a_t, r0, q),
                                start=(q == 0),
                                stop=False,
                                perf_mode=DR,
                            )
                        nc.tensor.matmul(
                            pst[:, b, :_XN],
                            w8_t[:],
                            a_t[:, r0 * _WP + _TAPOFF[_SINGLE] :][:, :_XN],
                            start=False,
                            stop=True,
                        )
                    # epilogue: out = (psum * s) + (x + t), valid cols only
                    # (one stt per bank: walrus caps APs at partition + 2 dims)
                    sl = slice(p * 2 * _CN, (p + 1) * 2 * _CN)
                    out_t = opool.tile([_P, 2 * _CN], f32, name="out_t", tag="o")
                    for b in range(2):
                        psv = pst[:, b, :_XN].rearrange(
                            "c (h w) -> c h w", w=_WP
                        )[:, :, :_W]
                        bs = slice(b * _CN, (b + 1) * _CN)
                        out_v = out_t[:, bs].rearrange("c (h w) -> c h w", h=_CH)
                        xp_v = xp_tiles[n][:, (2 * p + b) * _CN :][
                            :, :_CN
                        ].rearrange("c (h w) -> c h w", h=_CH)
                        nc.vector.scalar_tensor_tensor(
                            out_v, psv, s_t[:, 0:1], xp_v, MULT, ADD
                        )
                    nc.sync.dma_start(o_d[n, :, sl], out_t[:])

    nc.compile()
    return nc


def _get_program():
    if "nc" not in _cache:
        _cache["nc"] = _build_program()
    return _cache["nc"]


def _prep_inputs(x, weight, bias, gamma, beta, running_mean, running_var):
    # per-core batch shards
    xs = np.ascontiguousarray(
        np.asarray(x, dtype=np.float32).reshape(_NCORES, _NPI, _C, _NPIX)
    )
    # sign(weight) as [C, tap, P]; DoubleRow pairs [C, 4, 2, P] + single [C, P]
    wb = np.sign(np.asarray(weight, dtype=np.float32))  # [P, C, 3, 3]
    wT = np.ascontiguousarray(wb.transpose(1, 2, 3, 0).reshape(_C, 9, _P))
    fp8 = ml_dtypes.float8_e4m3
    wdr = np.ascontiguousarray(wT[:, :8, :].reshape(_C, 4, 2, _P)).astype(fp8)
    w8 = np.ascontiguousarray(wT[:, 8, :]).astype(fp8)
    inv = np.asarray(gamma, dtype=np.float64) / np.sqrt(
        np.asarray(running_var, dtype=np.float64) + _BN_EPS
    )
    shift = (
        np.asarray(bias, dtype=np.float64) * inv
        + np.asarray(beta, dtype=np.float64)
        - np.asarray(running_mean, dtype=np.float64) * inv
    )
    s = inv.astype(np.float32).reshape(_P, 1)
    t = shift.astype(np.float32).reshape(_P, 1)
    return [
        {"x": xs[i], "wdr": wdr, "w8": w8, "s": s, "t": t}
        for i in range(_NCORES)
    ]


def _run(inputs, trace=False, trace_cores=None):
    from concourse.bass_utils import run_bass_kernel_spmd

    nc = _get_program()
    in_maps = _prep_inputs(**inputs)
    res = run_bass_kernel_spmd(
        nc,
        in_maps,
        list(range(_NCORES)),
        trace=trace,
        trace_cores=trace_cores,
    )
    out = np.stack([res.results[i]["o"] for i in range(_NCORES)], axis=0)
    out = out.reshape(_N, _P, _H, _W).astype(np.float32, copy=False)
    return out, res


def kernel(**inputs):
    out, _ = _run(inputs, trace=False)
    return out


# revision 5
# speedup vs baseline: 2.9007x; 2.9007x over previous
"""Binary-conv BasicBlock (sign-act 3x3 binary conv + BN(eval) + residual).

Full shapes: x (32,128,56,56) f32, weight (128,128,3,3), BN params (128,).
Strategy: data-parallel over batch N across 8 NeuronCores (4 images/core).
Per image on-device:
  - sign(x) on ScalarE into a zero-padded bf16 tile (58x58 rows, flat);
    +/-1 exact in bf16, integer partial sums exact in fp32 PSUM -> conv
    bit-exact.  (fp8 DoubleRow was tried and measured: DR matmuls stream at
    1 elem/cycle on this HW, i.e. no speedup over 9 plain bf16 taps.)
  - conv = 9 shifted matmuls per 7-row chunk (N=392, one PSUM bank),
    chunk-major over 4 pair-tiles (2 banks each) so epilogues and stores
    overlap the next chunk's matmuls.
  - epilogue on VectorE: out = (psum * s) + (x + t) via scalar_tensor_tensor;
    (x + t) precomputed on ScalarE (GpSimd tensor_scalar measured 14ns/elem —
    useless), per-pair DMA stores.
  - x loads split in halves w/ split sign for fast pipeline start; warmup
    matmuls keep PE HAM un-throttled through the initial DMA wait.
"""

import numpy as np
import ml_dtypes

_N, _C, _H, _W = 32, 128, 56, 56
_P = 128
_NCORES = 8
_NPI = _N // _NCORES  # images per core
_HP, _WP = _H + 2, _W + 2
_NPIX = _H * _W
_APAD = _HP * _WP
_BN_EPS = 1e-5
_CH = 7               # output rows per PSUM bank chunk
_NCH = _H // _CH      # 8 chunks per image
_NPAIR = _NCH // 2    # 4 psum pair-tiles (2 banks each) per image
_CN = _CH * _W        # 392 elems per chunk

_cache = {}


def _build_program():
    import concourse.bacc as bacc
    import concourse.mybir as mybir
    import concourse.tile as tile

    f32 = mybir.dt.float32
    bf16 = mybir.dt.bfloat16

    nc = bacc.Bacc("TRN2", target_bir_lowering=False, debug=False)

    x_d = nc.dram_tensor("x", [_NPI, _C, _NPIX], f32, kind="ExternalInput")
    w_d = nc.dram_tensor("w", [_C, 9, _P], bf16, kind="ExternalInput")
    s_d = nc.dram_tensor("s", [_P, 1], f32, kind="ExternalInput")
    t_d = nc.dram_tensor("t", [_P, 1], f32, kind="ExternalInput")
    o_d = nc.dram_tensor("o", [_NPI, _P, _NPIX], f32, kind="ExternalOutput")

    SIGN = mybir.ActivationFunctionType.Sign
    IDENT = mybir.ActivationFunctionType.Identity
    MULT, ADD = mybir.AluOpType.mult, mybir.AluOpType.add
    HROWS = _H // 2  # 28 rows per x half

    with tile.TileContext(nc) as tc:
        with (
            tc.tile_pool(name="const", bufs=1) as cpool,
            tc.tile_pool(name="xin", bufs=3) as xpool,
            tc.tile_pool(name="apad", bufs=1) as apool,
            tc.tile_pool(name="outp", bufs=4) as opool,
            tc.tile_pool(name="ps", bufs=4, space="PSUM") as pspool,
        ):
            # Warmup source: tiny zero tile; matmuls on it keep the PE busy
            # (HAM stays at 8/8) while the first image loads.
            dummy = cpool.tile([_C, _P], bf16)
            nc.any.memset(dummy[:], 0.0)

            x_tiles = [None] * _NPI
            xp_tiles = [None] * _NPI

            def load_x(n):
                x_t = xpool.tile([_C, _NPIX], f32, name="x_t", tag="x")
                for h in range(2):
                    nc.sync.dma_start(
                        x_t[:, h * HROWS * _W : (h + 1) * HROWS * _W],
                        x_d[n, :, h * HROWS * _W : (h + 1) * HROWS * _W],
                    )
                x_tiles[n] = x_t

            load_x(0)

            wt = cpool.tile([_C, 9, _P], bf16)
            nc.sync.dma_start(wt[:], w_d[:])
            s_t = cpool.tile([_P, 1], f32)
            nc.sync.dma_start(s_t[:], s_d[:])
            t_t = cpool.tile([_P, 1], f32)
            nc.sync.dma_start(t_t[:], t_d[:])

            # Two persistent padded sign tiles; zeroed once (borders stay 0),
            # only the 56x56 interior is rewritten per image.
            a_tiles = []
            for i in range(2):
                a_t = apool.tile([_C, _APAD], bf16, name=f"apad{i}", tag=f"apad{i}")
                nc.any.memset(a_t[:], 0.0)
                a_tiles.append(a_t)

            def stage_img(n):
                """DMA x(n), then per half: sign -> a-pad, x+t -> xp (ScalarE)."""
                x_v = x_tiles[n][:].rearrange("c (h w) -> c h w", h=_H)
                a_v = a_tiles[n % 2][:].rearrange("c (h w) -> c h w", w=_WP)
                xp = xpool.tile([_C, _NPIX], f32, name="xp_t", tag="xp")
                xp_tiles[n] = xp
                for h in range(2):
                    r = h * HROWS
                    nc.scalar.activation(
                        a_v[:, 1 + r : 1 + r + HROWS, 1 : _W + 1],
                        x_v[:, r : r + HROWS, :],
                        SIGN,
                    )
                    nc.scalar.activation(
                        xp[:, r * _W : (r + HROWS) * _W],
                        x_tiles[n][:, r * _W : (r + HROWS) * _W],
                        IDENT,
                        bias=t_t[:, 0:1],
                    )

            stage_img(0)

            # PE warmup while image-0 DMA+sign are in flight (start/stop=True;
            # results discarded when the real group restarts the bank).
            warm_ps = pspool.tile([_P, 2, 512], f32, name="warm_ps", tag="ps")
            for i in range(72):
                nc.tensor.matmul(
                    warm_ps[:, i % 2, :128],
                    dummy[:],
                    dummy[:],
                    start=True,
                    stop=True,
                )

            for n in range(_NPI):
                # Emit next image's staging ahead of this image's epilogue so
                # the in-order ScalarE stream never stalls next matmuls.
                if n + 1 < _NPI:
                    load_x(n + 1)
                    stage_img(n + 1)
                a_t = a_tiles[n % 2]
                a_v = a_t[:].rearrange("c (h w) -> c h w", w=_WP)

                for p in range(_NPAIR):
                    pst = pspool.tile([_P, 2, 512], f32, name="pst", tag="ps")
                    for b in range(2):
                        c = 2 * p + b
                        r0 = c * _CH
                        for tp in range(9):
                            kh, kw = tp // 3, tp % 3
                            nc.tensor.matmul(
                                pst[:, b, :_CN],
                                wt[:, tp, :],
                                a_v[:, r0 + kh : r0 + kh + _CH, kw : kw + _W],
                                start=(tp == 0),
                                stop=(tp == 8),
                            )
                    # epilogue: out = (psum * s) + (x + t), one stt per bank
                    sl = slice(p * 2 * _CN, (p + 1) * 2 * _CN)
                    out_t = opool.tile([_P, 2 * _CN], f32, name="out_t", tag="o")
                    for b in range(2):
                        bs = slice(b * _CN, (b + 1) * _CN)
                        nc.vector.scalar_tensor_tensor(
                            out_t[:, bs],
                            pst[:, b, :_CN],
                            s_t[:, 0:1],
                            xp_tiles[n][:, (2 * p + b) * _CN :][:, :_CN],
                            MULT,
                            ADD,
                        )
                    nc.sync.dma_start(o_d[n, :, sl], out_t[:])

    nc.compile()
    return nc


def _get_program():
    if "nc" not in _cache:
        _cache["nc"] = _build_program()
    return _cache["nc"]


def _prep_inputs(x, weight, bias, gamma, beta, running_mean, running_var):
    # per-core batch shards
    xs = np.ascontiguousarray(
        np.asarray(x, dtype=np.float32).reshape(_NCORES, _NPI, _C, _NPIX)
    )
    # sign(weight) as [C, tap, P] bf16 (lhsT per tap)
    wb = np.sign(np.asarray(weight, dtype=np.float32))  # [P, C, 3, 3]
    wT = np.ascontiguousarray(
        wb.transpose(1, 2, 3, 0).reshape(_C, 9, _P)
    ).astype(ml_dtypes.bfloat16)
    inv = np.asarray(gamma, dtype=np.float64) / np.sqrt(
        np.asarray(running_var, dtype=np.float64) + _BN_EPS
    )
    shift = (
        np.asarray(bias, dtype=np.float64) * inv
        + np.asarray(beta, dtype=np.float64)
        - np.asarray(running_mean, dtype=np.float64) * inv
    )
    s = inv.astype(np.float32).reshape(_P, 1)
    t = shift.astype(np.float32).reshape(_P, 1)
    return [
        {"x": xs[i], "w": wT, "s": s, "t": t} for i in range(_NCORES)
    ]


def _run(inputs, trace=False, trace_cores=None):
    from concourse.bass_utils import run_bass_kernel_spmd

    nc = _get_program()
    in_maps = _prep_inputs(**inputs)
    res = run_bass_kernel_spmd(
        nc,
        in_maps,
        list(range(_NCORES)),
        trace=trace,
        trace_cores=trace_cores,
    )
    out = np.stack([res.results[i]["o"] for i in range(_NCORES)], axis=0)
    out = out.reshape(_N, _P, _H, _W).astype(np.float32, copy=False)
    return out, res


def kernel(**inputs):
    out, _ = _run(inputs, trace=False)
    return out


# revision 7
# speedup vs baseline: 2.9037x; 1.0010x over previous
"""Binary-conv BasicBlock (sign-act 3x3 binary conv + BN(eval) + residual).

Full shapes: x (32,128,56,56) f32, weight (128,128,3,3), BN params (128,).
Strategy: data-parallel over batch N across 8 NeuronCores (4 images/core).
Per image on-device:
  - sign(x) on ScalarE into a zero-padded bf16 tile (58x58 rows, flat);
    +/-1 exact in bf16, integer partial sums exact in fp32 PSUM -> conv
    bit-exact.  (fp8 DoubleRow was tried and measured: DR matmuls stream at
    1 elem/cycle on this HW, i.e. no speedup over 9 plain bf16 taps.)
  - conv = 9 shifted matmuls per 7-row chunk (N=392, one PSUM bank),
    chunk-major over 4 pair-tiles (2 banks each) so epilogues and stores
    overlap the next chunk's matmuls.
  - epilogue on VectorE: out = (psum * s) + (x + t) via scalar_tensor_tensor;
    (x + t) precomputed on ScalarE (GpSimd tensor_scalar measured 14ns/elem —
    useless), per-pair DMA stores.
  - x loads split in halves w/ split sign for fast pipeline start; warmup
    matmuls keep PE HAM un-throttled through the initial DMA wait.
"""

import numpy as np
import ml_dtypes

_N, _C, _H, _W = 32, 128, 56, 56
_P = 128
_NCORES = 8
_NPI = _N // _NCORES  # images per core
_HP, _WP = _H + 2, _W + 2
_NPIX = _H * _W
_APAD = _HP * _WP
_BN_EPS = 1e-5
_CH = 7               # output rows per PSUM bank chunk
_NCH = _H // _CH      # 8 chunks per image
_NPAIR = _NCH // 2    # 4 psum pair-tiles (2 banks each) per image
_CN = _CH * _W        # 392 elems per chunk

_cache = {}


def _build_program():
    import concourse.bacc as bacc
    import concourse.mybir as mybir
    import concourse.tile as tile

    f32 = mybir.dt.float32
    bf16 = mybir.dt.bfloat16

    nc = bacc.Bacc("TRN2", target_bir_lowering=False, debug=False)

    x_d = nc.dram_tensor("x", [_NPI, _C, _NPIX], f32, kind="ExternalInput")
    w_d = nc.dram_tensor("w", [_C, 9, _P], bf16, kind="ExternalInput")
    s_d = nc.dram_tensor("s", [_P, 1], f32, kind="ExternalInput")
    t_d = nc.dram_tensor("t", [_P, 1], f32, kind="ExternalInput")
    o_d = nc.dram_tensor("o", [_NPI, _P, _NPIX], f32, kind="ExternalOutput")

    SIGN = mybir.ActivationFunctionType.Sign
    IDENT = mybir.ActivationFunctionType.Identity
    MULT, ADD = mybir.AluOpType.mult, mybir.AluOpType.add
    HROWS = _H // 2  # 28 rows per x half

    with tile.TileContext(nc) as tc:
        with (
            tc.tile_pool(name="const", bufs=1) as cpool,
            tc.tile_pool(name="xin", bufs=3) as xpool,
            tc.tile_pool(name="apad", bufs=1) as apool,
            tc.tile_pool(name="outp", bufs=4) as opool,
            tc.tile_pool(name="ps", bufs=4, space="PSUM") as pspool,
        ):
            # Warmup source: tiny zero tile; matmuls on it keep the PE busy
            # (HAM stays at 8/8) while the first image loads.
            dummy = cpool.tile([_C, _P], bf16)
            nc.vector.memset(dummy[:], 0.0)

            x_tiles = [None] * _NPI
            xp_tiles = [None] * _NPI

            def load_x(n, nsplit=2):
                x_t = xpool.tile([_C, _NPIX], f32, name="x_t", tag="x")
                rows = _H // nsplit
                for h in range(nsplit):
                    nc.sync.dma_start(
                        x_t[:, h * rows * _W : (h + 1) * rows * _W],
                        x_d[n, :, h * rows * _W : (h + 1) * rows * _W],
                    )
                x_tiles[n] = x_t

            load_x(0, nsplit=4)

            wt = cpool.tile([_C, 9, _P], bf16)
            nc.sync.dma_start(wt[:], w_d[:])
            s_t = cpool.tile([_P, 1], f32)
            nc.sync.dma_start(s_t[:], s_d[:])
            t_t = cpool.tile([_P, 1], f32)
            nc.sync.dma_start(t_t[:], t_d[:])

            # Two persistent padded sign tiles; zeroed once (borders stay 0),
            # only the 56x56 interior is rewritten per image.
            a_tiles = []
            for i in range(2):
                a_t = apool.tile([_C, _APAD], bf16, name=f"apad{i}", tag=f"apad{i}")
                nc.vector.memset(a_t[:], 0.0)
                a_tiles.append(a_t)

            def stage_img(n, nsplit=2):
                """After x(n) DMA, per slice: sign -> a-pad, x+t -> xp (ScalarE)."""
                x_v = x_tiles[n][:].rearrange("c (h w) -> c h w", h=_H)
                a_v = a_tiles[n % 2][:].rearrange("c (h w) -> c h w", w=_WP)
                xp = xpool.tile([_C, _NPIX], f32, name="xp_t", tag="xp")
                xp_tiles[n] = xp
                rows = _H // nsplit
                for h in range(nsplit):
                    r = h * rows
                    nc.scalar.activation(
                        a_v[:, 1 + r : 1 + r + rows, 1 : _W + 1],
                        x_v[:, r : r + rows, :],
                        SIGN,
                    )
                    nc.scalar.activation(
                        xp[:, r * _W : (r + rows) * _W],
                        x_tiles[n][:, r * _W : (r + rows) * _W],
                        IDENT,
                        bias=t_t[:, 0:1],
                    )

            stage_img(0, nsplit=4)

            # PE warmup while image-0 DMA+sign are in flight (start/stop=True;
            # results discarded when the real group restarts the bank).
            warm_ps = pspool.tile([_P, 2, 512], f32, name="warm_ps", tag="ps")
            for i in range(72):
                nc.tensor.matmul(
                    warm_ps[:, i % 2, :128],
                    dummy[:],
                    dummy[:],
                    start=True,
                    stop=True,
                )

            for n in range(_NPI):
                # Emit next image's staging ahead of this image's epilogue so
                # the in-order ScalarE stream never stalls next matmuls.
                if n + 1 < _NPI:
                    load_x(n + 1)
                    stage_img(n + 1)
                a_t = a_tiles[n % 2]
                a_v = a_t[:].rearrange("c (h w) -> c h w", w=_WP)

                for p in range(_NPAIR):
                    pst = pspool.tile([_P, 2, 512], f32, name="pst", tag="ps")
                    for b in range(2):
                        c = 2 * p + b
                        r0 = c * _CH
                        for tp in range(9):
                            kh, kw = tp // 3, tp % 3
                            nc.tensor.matmul(
                                pst[:, b, :_CN],
                                wt[:, tp, :],
                                a_v[:, r0 + kh : r0 + kh + _CH, kw : kw + _W],
                                start=(tp == 0),
                                stop=(tp == 8),
                            )
                    # epilogue: out = (psum * s) + (x + t), one stt per bank
                    sl = slice(p * 2 * _CN, (p + 1) * 2 * _CN)
                    out_t = opool.tile([_P, 2 * _CN], f32, name="out_t", tag="o")
                    for b in range(2):
                        bs = slice(b * _CN, (b + 1) * _CN)
                        nc.vector.scalar_tensor_tensor(
                            out_t[:, bs],
                            pst[:, b, :_CN],
                            s_t[:, 0:1],
                            xp_tiles[n][:, (2 * p + b) * _CN :][:, :_CN],
                            MULT,
                            ADD,
                        )
                    nc.sync.dma_start(o_d[n, :, sl], out_t[:])

    nc.compile()
    return nc


def _get_program():
    if "nc" not in _cache:
        _cache["nc"] = _build_program()
    return _cache["nc"]


def _prep_inputs(x, weight, bias, gamma, beta, running_mean, running_var):
    # per-core batch shards
    xs = np.ascontiguousarray(
        np.asarray(x, dtype=np.float32).reshape(_NCORES, _NPI, _C, _NPIX)
    )
    # sign(weight) as [C, tap, P] bf16 (lhsT per tap)
    wb = np.sign(np.asarray(weight, dtype=np.float32))  # [P, C, 3, 3]
    wT = np.ascontiguousarray(
        wb.transpose(1, 2, 3, 0).reshape(_C, 9, _P)
    ).astype(ml_dtypes.bfloat16)
    inv = np.asarray(gamma, dtype=np.float64) / np.sqrt(
        np.asarray(running_var, dtype=np.float64) + _BN_EPS
    )
    shift = (
        np.asarray(bias, dtype=np.float64) * inv
        + np.asarray(beta, dtype=np.float64)
        - np.asarray(running_mean, dtype=np.float64) * inv
    )
    s = inv.astype(np.float32).reshape(_P, 1)
    t = shift.astype(np.float32).reshape(_P, 1)
    return [
        {"x": xs[i], "w": wT, "s": s, "t": t} for i in range(_NCORES)
    ]


def _run(inputs, trace=False, trace_cores=None):
    from concourse.bass_utils import run_bass_kernel_spmd

    nc = _get_program()
    in_maps = _prep_inputs(**inputs)
    res = run_bass_kernel_spmd(
        nc,
        in_maps,
        list(range(_NCORES)),
        trace=trace,
        trace_cores=trace_cores,
    )
    out = np.stack([res.results[i]["o"] for i in range(_NCORES)], axis=0)
    out = out.reshape(_N, _P, _H, _W).astype(np.float32, copy=False)
    return out, res


def kernel(**inputs):
    out, _ = _run(inputs, trace=False)
    return out
